# revision 29
# baseline (speedup 1.0000x reference)
"""DKT (Deep Knowledge Tracing) accumulate-concat model on 8 Trainium2 cores.

Model (per example): one-hot interactions x[t] (2S=1024), query one-hots q,
  emb   = x @ W_emb + b_emb
  count = cumulative count state (x one-hot => plain cumsum over t)
  z     = [emb, log1p(count), log1p(delta)]
  h     = LSTM(z)                      (Keras gates i,f,g,o; unit forget bias)
  y     = sum(sigmoid(h @ W_out + b_out) * q, -1)

Sharding: data-parallel over batch. 8 cores x 8 examples. Weights replicated.

Device algorithm per core (B'=8 examples), v2:
  Phase A (prologue): cast U/W_out/W_emb/biases to fp16. U and W_lstm gate
    columns are permuted to (f,i,g,o) so phase-2 activations fire in
    psum-completion order.
  Phase B (upfront): per example, count cumsum over full T as x^T @ triu
    (exact in fp16), log1p via ACT Ln (bias=1) -> L, xT via column diff;
    L and xT streamed to DRAM scratch (the Ln act table cannot coexist with
    the sigmoid table, so all Ln work happens before the recurrence).
  Phase C (per 128-step t-block): zT = [embT(2); L(8)] tiles; WzT = W^T z +
    bias via (m,kc)-ordered matmuls, each W tile loaded once and reused by
    all 8 examples; psum -> wz fp16 copies carry the bias via tensor_scalar
    adds spread over DVE/Pool/ACT. Block 0 runs upfront (fused with the
    fp32->fp16 W cast + scratch store + bias_g computation); blocks >=1
    are interleaved into the recurrence's idle engine windows.
  Phase D (recurrence): per step, wz[t] is pre-injected into the gate psums
    via identity-stationary matmuls (start=True) during the previous step's
    pointwise chain, then 64 ldweights+matmul pairs accumulate U^T h.
    Activations fire per-gate as psums complete (f first); c/h chain:
    cf=sf*c, ig=si*tg, c=cf+ig, th=tanh(c), h=so*th (fp16 -> ring).
  Phase E (fused output, every 16 steps): s = h @ W_out + b_out using the
    h-ring as stationary, y = accum(sigmoid(s) * q) via tensor_tensor_reduce.

Output DRAM tensor is [T, 8] (t-major); host transposes.
"""

import os
import sys

sys.path.insert(0, "/opt/trn_rl_repo")

KSTOP = int(os.environ.get("KSTOP", "9"))  # debug: stop after phase N
INTERLEAVE = os.environ.get("NO_INTERLEAVE", "0") != "1"

import numpy as np

import concourse.bass as bass
import concourse.tile as tile
from concourse import bacc, mybir
from concourse.bass_utils import run_bass_kernel_spmd

F32 = mybir.dt.float32
F16 = mybir.dt.float16
AF = mybir.ActivationFunctionType
ALU = mybir.AluOpType

N_CORES = 8
B_FULL, T_FULL, S = 64, 512, 512
S2 = 2 * S          # 1024 one-hot dim
DE = 256            # emb dim
H = 512             # lstm hidden
G4 = 4 * H          # 2048 gate cols
BP = 8              # examples per core
TB = 128            # t-block size

# gate-column chunk permutation: keras order i,f,g,o -> ours f,i,g,o
PERM = [4, 5, 6, 7, 0, 1, 2, 3, 8, 9, 10, 11, 12, 13, 14, 15]


def _build(T=T_FULL):
    KT = T // 128
    nc = bacc.Bacc("TRN2", target_bir_lowering=False, debug=False)

    x_h = nc.dram_tensor("x", [BP, T, S2], F32, kind="ExternalInput")
    d_h = nc.dram_tensor("delta", [BP, T], F32, kind="ExternalInput")
    q_h = nc.dram_tensor("q", [BP, T, S], F32, kind="ExternalInput")
    we_h = nc.dram_tensor("W_emb", [S2, DE], F32, kind="ExternalInput")
    be_h = nc.dram_tensor("b_emb", [DE], F32, kind="ExternalInput")
    wl_h = nc.dram_tensor("W_lstm", [S2 + DE + 1, G4], F32, kind="ExternalInput")
    ul_h = nc.dram_tensor("U_lstm", [H, G4], F32, kind="ExternalInput")
    bl_h = nc.dram_tensor("b_lstm", [G4], F32, kind="ExternalInput")
    wo_h = nc.dram_tensor("W_out", [H, S], F32, kind="ExternalInput")
    bo_h = nc.dram_tensor("b_out", [S], F32, kind="ExternalInput")
    y_h = nc.dram_tensor("y", [T, BP], F32, kind="ExternalOutput")

    # scratch in DRAM
    wlh_h = nc.dram_tensor("wl_f16", [S2 + DE, G4], F16, kind="Internal")
    l_h = nc.dram_tensor("l_f16", [BP, 8, 128, T], F16, kind="Internal")
    xt_h = nc.dram_tensor("xt_f16", [BP, 8, 128, T], F16, kind="Internal")

    tri_h = nc.inline_tensor(np.triu(np.ones((T, T), np.float16)), name="triu")
    id_h = nc.inline_tensor(np.eye(128, dtype=np.float16), name="ident")

    aps = dict(
        x=x_h.ap(), d=d_h.ap(), q=q_h.ap(), we=we_h.ap(), be=be_h.ap(),
        wl=wl_h.ap(), ul=ul_h.ap(), bl=bl_h.ap(), wo=wo_h.ap(),
        bo=bo_h.ap(), y=y_h.ap(), wlh=wlh_h.ap(), lh=l_h.ap(),
        xth=xt_h.ap(), tri=tri_h.ap(), ident=id_h.ap(),
    )

    with tile.TileContext(nc) as tc:
        _kernel_body(nc, tc, T, KT, aps)
    nc.compile()
    return nc


def _kernel_body(nc, tc, T, KT, aps):
    from contextlib import ExitStack

    x, d, q = aps["x"], aps["d"], aps["q"]
    we, be, wl, ul, bl = aps["we"], aps["be"], aps["wl"], aps["ul"], aps["bl"]
    wo, bo, y = aps["wo"], aps["bo"], aps["y"]
    wlh, lh, xth = aps["wlh"], aps["lh"], aps["xth"]
    tri, ident = aps["tri"], aps["ident"]

    NB = T // TB  # number of t-blocks

    ctx = ExitStack()
    with ctx:
        # ---------------- persistent SBUF ----------------
        per = ctx.enter_context(tc.tile_pool(name="persist", bufs=1))
        wz_pool = ctx.enter_context(tc.tile_pool(name="wz", bufs=1))

        wzb = [wz_pool.tile([128, 16, BP, TB], F16, name=f"wzb{b}")
       for b in range(T // TB)]                  # 128KB/part total
        u_sb = per.tile([128, 4, G4], F16)               # 16KB/part
        wo_sb = per.tile([128, 4, S], F16)               # 4KB/part
        wemb = per.tile([128, 8, DE], F16)               # 4KB/part
        id_sb = per.tile([128, 128], F16)
        ldt = per.tile([1, BP, T], F16)
        bembT = per.tile([128, 2], F32)
        blstm = per.tile([128, 16], F32)
        bias_g = per.tile([128, 16], F32)
        bout16 = per.tile([1, S], F16)
        ones1 = per.tile([1, 128], F16)
        wd = per.tile([1, G4], F16)
        hring = per.tile([128, 33, 4, BP], F16)          # 2.1KB/part
        hringB = per.tile([128, 4, 33, BP], F16)         # phase-E layout
        cbuf = per.tile([128, 2, 4, BP], F32)
        sf = per.tile([128, 4, BP], F32)
        si = per.tile([128, 4, BP], F32)
        tg = per.tile([128, 4, BP], F32)
        so = per.tile([128, 4, BP], F32)
        igt = per.tile([128, 4, BP], F32)
        cft = per.tile([128, 4, BP], F32)
        th = per.tile([128, 4, BP], F32)

        nc.vector.memset(ones1, 1.0)
        nc.vector.memset(hring[:, 0, :, :], 0.0)
        nc.vector.memset(hringB[:, :, 0, :], 0.0)
        nc.vector.memset(cbuf[:, 1, :, :], 0.0)  # c_prev for t=0

        # ---------------- phase A: prologue ----------------
        with tc.tile_pool(name="pa", bufs=1) as pa:
            nc.sync.dma_start(out=id_sb, in_=ident[0:128, :])
            for ko in range(4):
                t32 = pa.tile([128, G4], F32, tag="u32")
                nc.sync.dma_start(out=t32, in_=ul[128 * ko:128 * (ko + 1), :])
                for m in range(16):
                    nc.scalar.activation(
                        u_sb[:, ko, 128 * m:128 * (m + 1)],
                        t32[:, 128 * PERM[m]:128 * (PERM[m] + 1)], AF.Copy)
            for ko in range(4):
                t32 = pa.tile([128, S], F32, tag="wo32")
                nc.sync.dma_start(out=t32, in_=wo[128 * ko:128 * (ko + 1), :])
                nc.scalar.activation(wo_sb[:, ko, :], t32, AF.Copy)
            for mc in range(8):
                t32 = pa.tile([128, DE], F32, tag="we32")
                nc.sync.dma_start(out=t32, in_=we[128 * mc:128 * (mc + 1), :])
                nc.scalar.activation(wemb[:, mc, :], t32, AF.Copy)
            t32 = pa.tile([128, 2], F32, tag="be32")
            nc.sync.dma_start(
                out=t32,
                in_=bass.AP(tensor=be.tensor, offset=be.offset,
                            ap=[[1, 128], [128, 2]]))
            nc.vector.tensor_copy(bembT, t32)
            nc.sync.dma_start(
                out=blstm,
                in_=bass.AP(tensor=bl.tensor, offset=bl.offset,
                            ap=[[1, 128], [128, 16]]))
            t32 = pa.tile([1, S], F32, tag="bo32")
            nc.sync.dma_start(out=t32, in_=bo[None, :])
            nc.scalar.activation(bout16, t32, AF.Copy)
            t32 = pa.tile([1, G4], F32, tag="wd32")
            nc.sync.dma_start(out=t32, in_=wl[1280:1281, :])
            for m in range(16):
                nc.scalar.activation(
                    wd[:, 128 * m:128 * (m + 1)],
                    t32[:, 128 * PERM[m]:128 * (PERM[m] + 1)], AF.Copy)

        if KSTOP < 1:
            return

        # ---------------- phase B: counts + log1p + xT upfront ----------------
        with tc.tile_pool(name="pbx", bufs=1) as pbx, \
             tc.tile_pool(name="pbl", bufs=1) as pbl, \
             tc.tile_pool(name="pbt", bufs=1) as pbt, \
             tc.tile_pool(name="pbps", bufs=2, space="PSUM") as pbps:
            tri_sb = pbt.tile([128, KT, T], F16)
            for kt in range(KT):
                nc.sync.dma_start(out=tri_sb[:, kt, :],
                                  in_=tri[128 * kt:128 * (kt + 1), :])
            for e8 in range(8):
                ld32 = pbt.tile([1, 1, T], F32, tag="ld32",
                                name=f"ld32_{e8}")
                nc.sync.dma_start(
                    out=ld32,
                    in_=bass.AP(tensor=d.tensor, offset=d.offset + e8 * T,
                                ap=[[0, 1], [T, 1], [1, T]]))
                nc.scalar.activation(ldt[:, e8:e8 + 1, :], ld32,
                                     AF.Ln, bias=1.0)
            # bias_g[:, m] = (b_emb @ W1)[m-chunk] + b_lstm[m-chunk] (fp32 mm)
            for m in range(16):
                bias_ps = pbps.tile([128, 1], F32, tag="bias",
                                    name=f"biasps_{m}")
                for kc in range(2):
                    w32b = pbl.tile([128, 128], F32, tag="w32b")
                    nc.sync.dma_start(
                        out=w32b,
                        in_=wl[128 * kc:128 * (kc + 1),
                               128 * PERM[m]:128 * (PERM[m] + 1)])
                    nc.tensor.matmul(bias_ps, w32b, bembT[:, kc:kc + 1],
                                     start=(kc == 0), stop=(kc == 1))
                nc.vector.tensor_add(bias_g[:, m:m + 1], bias_ps,
                                     blstm[:, PERM[m]:PERM[m] + 1])

            for ex in range(BP):
                x16 = []
                for kt in range(KT):
                    x32 = pbx.tile([128, S2], F32, tag="x32",
                                   name=f"x32_{ex}_{kt}")
                    nc.sync.dma_start(
                        out=x32, in_=x[ex, 128 * kt:128 * (kt + 1), :])
                    t16 = pbx.tile([128, S2], F16, tag=f"x16_{kt}",
                                   name=f"x16_{ex}_{kt}")
                    nc.scalar.activation(t16, x32, AF.Copy)
                    x16.append(t16)
                lall = pbl.tile([128, 8, T], F16, tag="lall",
                                name=f"lall_{ex}")
                xall = pbl.tile([128, 8, T], F16, tag="xall",
                                name=f"xall_{ex}")
                for mc in range(8):
                    cnt_ps = pbps.tile([128, T], F32, tag="cnt",
                                       name=f"cnt_{ex}_{mc}")
                    for kt in range(KT):
                        nc.tensor.matmul(
                            cnt_ps, x16[kt][:, 128 * mc:128 * (mc + 1)],
                            tri_sb[:, kt, :],
                            start=(kt == 0), stop=(kt == KT - 1))
                    nc.scalar.activation(lall[:, mc, :], cnt_ps, AF.Ln,
                                         bias=1.0)
                    csb = pbl.tile([128, T], F16, tag="csb")
                    nc.vector.tensor_copy(csb, cnt_ps)
                    nc.vector.tensor_copy(xall[:, mc, 0:1], csb[:, 0:1])
                    nc.vector.tensor_sub(xall[:, mc, 1:T], csb[:, 1:T],
                                         csb[:, 0:T - 1])
                nc.sync.dma_start(
                    out=bass.AP(tensor=lh.tensor,
                                offset=lh.offset + ex * 8 * 128 * T,
                                ap=[[T, 128], [128 * T, 8], [1, T]]),
                    in_=lall)
                nc.sync.dma_start(
                    out=bass.AP(tensor=xth.tensor,
                                offset=xth.offset + ex * 8 * 128 * T,
                                ap=[[T, 128], [128 * T, 8], [1, T]]),
                    in_=xall)

        if KSTOP < 2:
            return

        # ---------------- stream pools (allocated after phase B frees) -----
        ztp = ctx.enter_context(tc.tile_pool(name="ztp", bufs=1))
        zt_all = ztp.tile([128, BP, 10, TB], F16, name="zt_all")
        gps_pool = ctx.enter_context(
            tc.tile_pool(name="gps", bufs=1, space="PSUM"))
        wzps_pool = ctx.enter_context(
            tc.tile_pool(name="wzps", bufs=1, space="PSUM"))
        embps_pool = ctx.enter_context(
            tc.tile_pool(name="embps", bufs=1, space="PSUM"))
        outps_pool = ctx.enter_context(
            tc.tile_pool(name="outps", bufs=1, space="PSUM"))
        wstr_pool = ctx.enter_context(tc.tile_pool(name="wstr", bufs=1))
        xstr_pool = ctx.enter_context(tc.tile_pool(name="xstr", bufs=1))
        rd_pool = ctx.enter_context(tc.tile_pool(name="rd", bufs=1))

        gfi = gps_pool.tile([128, 8, BP], F32, tag="gfi", name="gfi")
        gg = gps_pool.tile([128, 4, BP], F32, tag="gg", name="gg")
        go = gps_pool.tile([128, 4, BP], F32, tag="go", name="go")
        sfi = per.tile([128, 8, BP], F32)

        copy_rr = [0]

        def psum_copy_bias(dst, src, bias_ap):
            """dst(fp16) = src(psum fp32) + bias  (per-partition bias)."""
            r = copy_rr[0] % 2
            copy_rr[0] += 1
            if r == 0:
                nc.vector.tensor_scalar(dst, src, bias_ap, None, ALU.add)
            else:
                nc.scalar.activation(dst, src, AF.Identity, bias=bias_ap)

        # ---- emit one t-block of phase-C work as a list of chunklets ----
        def block_chunklets(b):
            t0 = b * TB
            chunks = []
            xt16 = {}
            wz_ps = {}
            bias_ps = {}

            def prep_l(pr):
                def f():
                    xt16[pr] = xstr_pool.tile(
                        [128, 2, 8, TB], F16, tag=f"xt{pr % 2}",
                        name=f"xt16_{b}_{pr}")
                    for i in range(2):
                        ex = 2 * pr + i
                        nc.sync.dma_start(
                            out=zt_all[:, ex, 2:10, :],
                            in_=bass.AP(
                                tensor=lh.tensor,
                                offset=lh.offset + ex * 8 * 128 * T + t0,
                                ap=[[T, 128], [128 * T, 8], [1, TB]]))
                        nc.sync.dma_start(
                            out=xt16[pr][:, i, :, :],
                            in_=bass.AP(
                                tensor=xth.tensor,
                                offset=xth.offset + ex * 8 * 128 * T + t0,
                                ap=[[T, 128], [128 * T, 8], [1, TB]]))
                return f

            def emb_pair(pr, m2):
                def f():
                    eps = embps_pool.tile([128, 2, TB], F32, tag="emb",
                                          name=f"embps_{b}_{pr}_{m2}")
                    for mc in range(8):
                        nc.tensor.matmul(
                            eps,
                            wemb[:, mc, 128 * m2:128 * (m2 + 1)],
                            xt16[pr][:, :, mc, :],
                            start=(mc == 0), stop=(mc == 7))
                    psum_copy_bias(zt_all[:, 2 * pr:2 * pr + 2, m2, :],
                                   eps, bembT[:, m2:m2 + 1])
                return f

            for pr in range(4):
                chunks.append(prep_l(pr))
                chunks.append(emb_pair(pr, 0))
                chunks.append(emb_pair(pr, 1))

            PF = 3  # W-tile DMA prefetch depth (in (m,kc) groups)
            wt_tiles = {}

            def w_fetch(idx):
                m, kc = divmod(idx, 10)
                if b == 0:
                    w32 = wstr_pool.tile([128, 128], F32,
                                         tag=f"w32_{idx % 2}",
                                         name=f"w32_{b}_{idx}")
                    nc.sync.dma_start(
                        out=w32,
                        in_=wl[128 * kc:128 * (kc + 1),
                               128 * PERM[m]:128 * (PERM[m] + 1)])
                    wt = wstr_pool.tile([128, 128], F16,
                                        tag=f"wt{idx % (PF + 2)}",
                                        name=f"wt_{b}_{idx}")
                    nc.scalar.activation(wt, w32, AF.Copy)
                    nc.sync.dma_start(
                        out=wlh[128 * kc:128 * (kc + 1),
                                128 * m:128 * (m + 1)],
                        in_=wt)
                else:
                    wt = wstr_pool.tile([128, 128], F16,
                                        tag=f"wt{idx % (PF + 2)}",
                                        name=f"wt_{b}_{idx}")
                    nc.sync.dma_start(
                        out=wt,
                        in_=wlh[128 * kc:128 * (kc + 1),
                                128 * m:128 * (m + 1)])
                wt_tiles[idx] = wt

            def wz_k(m, kc):
                def f():
                    idx = 10 * m + kc
                    if idx == 0:
                        for j in range(min(PF + 2, 160)):
                            w_fetch(j)
                    elif idx + PF + 2 <= 160:
                        w_fetch(idx + PF + 1)
                    wt = wt_tiles.pop(idx)
                    if kc == 0:
                        for gidx in range(2):
                            wz_ps[(m, gidx)] = wzps_pool.tile(
                                [128, 4, TB], F32, tag=f"wz{gidx}",
                                name=f"wzps_{b}_{m}_{gidx}")
                    for gidx in range(2):
                        nc.tensor.matmul(
                            wz_ps[(m, gidx)], wt,
                            zt_all[:, 4 * gidx:4 * (gidx + 1), kc, :],
                            start=(kc == 0), stop=False)
                return f

            def wz_fin(m, egrp):
                def f():
                    nc.tensor.matmul(
                        wz_ps[(m, egrp)],
                        wd[:, 128 * m:128 * (m + 1)],
                        ldt[:, 4 * egrp:4 * (egrp + 1), t0:t0 + TB],
                        start=False, stop=True)
                    psum_copy_bias(
                        wzb[b][:, m, 4 * egrp:4 * (egrp + 1), :],
                        wz_ps[(m, egrp)],
                        bias_g[:, m:m + 1])
                return f

            for m in range(16):
                for kc in range(10):
                    chunks.append(wz_k(m, kc))
                chunks.append(wz_fin(m, 0))
                chunks.append(wz_fin(m, 1))
            return chunks

        # block 0 upfront; later blocks interleave into the step stream
        for f in block_chunklets(0):
            f()
        pending = []
        if NB > 1:
            if INTERLEAVE and KSTOP >= 4:
                for b in range(1, NB):
                    pending.append(block_chunklets(b))
            else:
                for b in range(1, NB):
                    for f in block_chunklets(b):
                        f()

        if KSTOP < 3:
            return

        # ---------------- phase D: recurrence ----------------
        def inject(t):
            wzt = wzb[t // TB]
            tl = t % TB
            nc.tensor.matmul(gfi, id_sb, wzt[:, 0:8, :, tl],
                             start=True, stop=False)
            nc.tensor.matmul(gg, id_sb, wzt[:, 8:12, :, tl],
                             start=True, stop=False)
            nc.tensor.matmul(go, id_sb, wzt[:, 12:16, :, tl],
                             start=True, stop=False)

        inject(0)

        pe_q = []

        def pe_pieces(tb0):
            sl0 = 1 + (tb0 % 32)
            st = {}

            def p0():
                st['q'] = rd_pool.tile([128, S], F32, tag="qt",
                                       name=f"qt_{tb0}")
                nc.sync.dma_start(
                    out=st['q'],
                    in_=bass.AP(tensor=q.tensor,
                                offset=q.offset + tb0 * S,
                                ap=[[S, 16], [T * S, BP], [1, S]]))
                st['s'] = outps_pool.tile([128, S], F32, tag="sps",
                                          name=f"sps_{tb0}")
                for ko in range(2):
                    nc.tensor.matmul(
                        st['s'], hringB[:, ko, sl0:sl0 + 16, :],
                        wo_sb[:, ko, :],
                        start=(ko == 0), stop=False)

            def p1():
                for ko in range(2, 4):
                    nc.tensor.matmul(
                        st['s'], hringB[:, ko, sl0:sl0 + 16, :],
                        wo_sb[:, ko, :],
                        start=False, stop=False)
                nc.tensor.matmul(st['s'], ones1, bout16,
                                 start=False, stop=True)

            def p2():
                st['sig'] = rd_pool.tile([128, S], F32, tag="sig",
                                         name=f"sig_{tb0}")
                nc.scalar.activation(st['sig'], st['s'], AF.Sigmoid)

            def p3():
                prod = rd_pool.tile([128, S], F32, tag="prod",
                                    name=f"prod_{tb0}")
                ycol = rd_pool.tile([128, 1], F32, tag="ycol",
                                    name=f"ycol_{tb0}")
                nc.vector.tensor_mul(prod, st['sig'], st['q'])
                nc.vector.tensor_reduce(ycol, prod, mybir.AxisListType.X,
                                        ALU.add)
                nc.sync.dma_start(out=y[tb0:tb0 + 16, :], in_=ycol)

            return [p0, p1, p2, p3]

        SPREAD = TB - 8
        for t in range(T):
            par = t % 2
            sl_prev = 1 + ((t - 1) % 32) if t > 0 else 0
            sl = 1 + (t % 32)
            cprev = cbuf[:, 1 - par, :, :]
            cnew = cbuf[:, par, :, :]

            for m in range(8):
                for ko in range(4):
                    nc.tensor.matmul(
                        gfi[:, m, :],
                        u_sb[:, ko, 128 * m:128 * (m + 1)],
                        hring[:, sl_prev, ko, :],
                        start=False, stop=(ko == 3))
            nc.scalar.activation(sfi, gfi, AF.Sigmoid)
            nc.vector.tensor_mul(cft, sfi[:, 0:4, :], cprev)
            for m in range(8, 12):
                for ko in range(4):
                    nc.tensor.matmul(
                        gg[:, m - 8, :],
                        u_sb[:, ko, 128 * m:128 * (m + 1)],
                        hring[:, sl_prev, ko, :],
                        start=False, stop=(ko == 3))
            nc.scalar.activation(tg, gg, AF.Tanh)
            nc.vector.tensor_mul(igt, sfi[:, 4:8, :], tg)
            nc.vector.tensor_add(cnew, cft, igt)
            nc.scalar.activation(th, cnew, AF.Tanh)
            for m in range(12, 16):
                for ko in range(4):
                    nc.tensor.matmul(
                        go[:, m - 12, :],
                        u_sb[:, ko, 128 * m:128 * (m + 1)],
                        hring[:, sl_prev, ko, :],
                        start=False, stop=(ko == 3))
            nc.scalar.activation(so, go, AF.Sigmoid)
            nc.vector.tensor_mul(hring[:, sl, :, :], so, th)
            nc.gpsimd.tensor_copy(hringB[:, :, sl, :],
                                  hring[:, sl, :, :])
            if t + 1 < T:
                inject(t + 1)

            # phase E: fused output, one piece per following step
            if t % 16 == 15 and KSTOP >= 5:
                if t == T - 1:
                    for f in pe_pieces(t - 15):
                        f()
                else:
                    pe_q.extend(pe_pieces(t - 15))
            if pe_q:
                pe_q.pop(0)()

            # interleaved phase-C chunklets producing block t//TB + 1; the
            # SPREAD margin guarantees each block's last write is emitted
            # well before the next block's first inject (emission order =
            # dependency order for Tile).
            if pending and t // TB < NB - 1:
                lst = pending[t // TB]
                n_total = len(lst)
                pos = min(t % TB, SPREAD)
                lo = (pos * n_total) // SPREAD
                hi = ((pos + 1) * n_total) // SPREAD if pos < SPREAD else n_total
                for i in range(lo, min(hi, n_total)):
                    lst[i]()



_CACHE = {}


def _get_nc(T=T_FULL):
    if T not in _CACHE:
        _CACHE[T] = _build(T)
    return _CACHE[T]


def kernel(x, delta, q, W_emb, b_emb, W_lstm, U_lstm, b_lstm, W_out, b_out):
    T = x.shape[1]
    nc = _get_nc(T)
    shared = dict(
        W_emb=np.ascontiguousarray(W_emb, np.float32),
        b_emb=np.ascontiguousarray(b_emb, np.float32),
        W_lstm=np.ascontiguousarray(W_lstm, np.float32),
        U_lstm=np.ascontiguousarray(U_lstm, np.float32),
        b_lstm=np.ascontiguousarray(b_lstm, np.float32),
        W_out=np.ascontiguousarray(W_out, np.float32),
        b_out=np.ascontiguousarray(b_out, np.float32),
    )
    in_maps = []
    for c in range(N_CORES):
        sl = slice(BP * c, BP * (c + 1))
        in_maps.append(dict(
            x=np.ascontiguousarray(x[sl], np.float32),
            delta=np.ascontiguousarray(np.asarray(delta)[sl, :, 0], np.float32),
            q=np.ascontiguousarray(q[sl], np.float32),
            **shared,
        ))
    res = run_bass_kernel_spmd(nc, in_maps, core_ids=list(range(N_CORES)))
    out = np.empty((x.shape[0], T, 1), np.float32)
    for c in range(N_CORES):
        out[BP * c:BP * (c + 1), :, 0] = res.results[c]["y"].T
    return out
